# revision 40
# baseline (speedup 1.0000x reference)
"""AttentionBlock (GroupNorm + single-head self-attention + residual) on
8 TRN2 NeuronCores.

Sharding: data-parallel over batch (4 images) x 2-way sequence-parallel
over query tokens => 8 cores, zero collectives. Each core receives one
full image x[b] [C=512, N=4096] (token columns rotated so that its own
2048 query tokens sit in columns 0..2047), computes GroupNorm + K/V over
all 4096 tokens (K/V duplicated across the 2 cores of a batch pair --
cheaper than an all-gather at this size: the pair-exchange variant was
measured and the collective's ~35-45us latency sits on the critical
path, a net loss), Q / attention / proj / residual for its 2048
queries, and returns y [512, 2048].

All matmuls run fp8e4m3 with DoubleRow (K=256 per instruction, ~1.8x
bf16-equivalent throughput).  Weights are quantized to fp8 on the host
(x16 prescale so w*16 sits in fp8's sweet spot; the 1/16 is folded into
the PSUM evictions).  Host also folds wp@bv into an effective bp, so the
V eviction is a single scaled copy.

On-chip layout ("channels on partitions"):
  t8 = groupnorm(x)            [c, n]   fp8   (ScalarE activation)
  xb = x + bp_eff              [c, nq]  f32   (residual staging)
  Q8 = (wq16 @ t8)/16 + bq     [d, nq]  fp8
  K8 = (wk16 @ t8)/16 + bk     [d, m]   fp8
  V8 = (t8^T @ wv16T)/16       [m, d]   fp8   (computed directly transposed)
  S^T[m, nq] = K8^T Q8         (PE, 2 DR matmuls per 128-key tile)
  E  = exp(S^T*SCALE + shift)  (ScalarE, PSUM->SBUF, fp8; shift=-ln 8)
  L[nq]   = ones^T @ E         (PE accumulate, = true_L/8)
  Ou[d,nq] = V8^T @ E          (PE accumulate, = true_A/8)
  o8 = Ou/64 (fp8);  rb = 4/L broadcast (recip of ones x (L/4))
  y  = (wp16 @ o8) * rb + xb   -> DMA out   [proj scale: 16/512 * 32 = 1]

Softmax skips the max-subtraction: logits are ~N(0,1) by construction.
DMA order matters: the 8MB x load is issued first on the SP queue (it
fans out across DMA engines); constants/weights ride the Activation
HWDGE queue so nothing serializes ahead of x.
"""

import sys

for _p in ("/opt/trn_rl_repo", "/opt/pypackages"):
    if _p not in sys.path:
        sys.path.append(_p)

import ml_dtypes
import numpy as np

import concourse.bass as bass
import concourse.tile as tile
from concourse import mybir
from concourse.bass_utils import run_bass_kernel_spmd
from concourse.vector_clock import ScopedClock

# ----------------------------------------------------------------------
# Problem constants (nn_AttentionBlock_24764781429183)
B, C, H, W = 4, 512, 64, 64
N = H * W              # 4096 tokens
NQ = N // 2            # 2048 query tokens per core
GROUPS = 32
GSIZE = C // GROUPS    # 16 channels per group
EPS = 1e-5
SCALE = 1.0 / float(np.sqrt(C))
CT = C // 128          # 4 channel tiles
MT = N // 128          # 32 key tiles
FB = 512               # matmul free-dim block
QB = NQ // FB          # 4 query blocks
NB = N // FB           # 8 token blocks
WS = 16.0              # host-side weight prescale before fp8 quantization

F32 = mybir.dt.float32
BF16 = mybir.dt.bfloat16
FP8 = mybir.dt.float8e4
DR = mybir.MatmulPerfMode.DoubleRow
IDENT = mybir.ActivationFunctionType.Identity
EXP = mybir.ActivationFunctionType.Exp
SQRT = mybir.ActivationFunctionType.Sqrt
MULT = mybir.AluOpType.mult
ADD = mybir.AluOpType.add
# exp(s*SCALE + EXP_SHIFT) = exp(s*SCALE)/8 — keeps E safely inside
# fp8e4m3 range (max 448) even for outlier logits; cancels in E/L.
EXP_SHIFT = -2.0794415416798357


# ----------------------------------------------------------------------
# This container's walrus build rejects >1 semaphore wait on one CTRL
# (Drain) instruction; split the Tile end-of-kernel drain waits across
# one-nop-per-wait instead.
def _patched_drain_and_barrier(self, tick_clock, wait_clock):
    nc = self.nc
    probe = nc.sync.nop(nofuse=True)
    wait_clock.add_sem_waits(probe.ins, ScopedClock({None: tick_clock.global_clock}))
    sync_info = probe.ins.sync_info
    waits = list(sync_info.on_wait or []) if sync_info is not None else []
    if sync_info is not None and len(waits) > 1:
        sync_info.on_wait = waits[:1]
        for w in waits[1:]:
            n = nc.sync.nop(nofuse=True)
            if n.ins.sync_info is None:
                n.ins.sync_info = type(sync_info)(on_wait=[w], on_update=[])
            else:
                n.ins.sync_info.on_wait = [w]
    nc.sync.drain()
    nc.all_engine_barrier()
    assert self.sems is not None
    popped = nc._tile_sem_poison_stack.pop()
    assert popped is self._sem_poison
    nc.clear_and_free_semaphores(list(self.sems.allocated().values()))
    nc.all_engine_barrier()


tile.TileContext._drain_and_barrier = _patched_drain_and_barrier


# Disk-cache compiled NEFFs by BIR hash — the bass_exec compile path
# bypasses libneuronxla's HLO-keyed cache, so without this every fresh
# process pays the full (~6 min) walrus compile.
def _install_neff_cache():
    import hashlib
    import os
    import shutil

    import concourse.bass2jax as bass2jax

    if getattr(bass2jax, "_neff_cache_installed", False):
        return
    orig = bass2jax.compile_bir_kernel

    def cached(bir_json, tmpdir, neff_name="file.neff"):
        cdir = os.environ.get("BASS_NEFF_CACHE", "/tmp/bass_neff_cache")
        os.makedirs(cdir, exist_ok=True)
        cpath = os.path.join(cdir, hashlib.sha256(bir_json).hexdigest()[:32] + ".neff")
        dst = os.path.join(tmpdir, neff_name)
        if os.path.exists(cpath):
            shutil.copy(cpath, dst)
            return dst
        out = orig(bir_json, tmpdir, neff_name=neff_name)
        try:
            shutil.copy(out, cpath)
        except OSError:
            pass
        return out

    bass2jax.compile_bir_kernel = cached
    bass2jax._neff_cache_installed = True


_install_neff_cache()


def _split_multi_waits(nc: bass.Bass, maxw: int = 1) -> None:
    """Walrus in this container rejects instructions carrying more than one
    semaphore wait. Hoist extra waits onto same-engine no-ops inserted
    right before the instruction (engine streams execute in block order,
    so the waits still gate the instruction)."""
    ctr = 0
    for fn in nc.m.functions:
        for bb in fn.blocks:
            out = []
            changed = False
            for inst in bb.instructions:
                si = inst.sync_info
                waits = list(si.on_wait) if (si is not None and si.on_wait) else []
                if len(waits) > maxw and inst.engine != mybir.EngineType.Unassigned:
                    keep = waits[-maxw:]
                    for i in range(0, len(waits) - maxw, maxw):
                        nop = mybir.InstNoOp(name=f"waitsplit-{ctr}")
                        ctr += 1
                        nop.engine = inst.engine
                        nop.sync_info = mybir.SyncInfo(
                            on_wait=waits[i : i + maxw], on_update=[]
                        )
                        out.append(nop)
                    si.on_wait = keep
                    inst.sync_info = si
                    changed = True
                out.append(inst)
            if changed:
                bb.instructions = out


# ----------------------------------------------------------------------
def _build_nc() -> bass.Bass:
    nc = bass.Bass()

    x_ext = nc.declare_dram_parameter("x", [C, N], F32, isOutput=False)
    # residual staging x_own + bp_eff, DMA'd straight into SBUF (keeps the
    # per-channel bias add off ScalarE during the GroupNorm phase)
    xres_ext = nc.declare_dram_parameter("xres", [C, NQ], F32, isOutput=False)
    w_ext = {
        k: nc.declare_dram_parameter(k, [128, CT * C], FP8, isOutput=False)
        for k in ("wq8", "wk8", "wv8", "wp8")
    }
    # packed per-channel vectors: bq | bk | bp_eff | gnw | gnb, each [128, CT]
    bias_ext = nc.declare_dram_parameter("biases", [128, 5 * CT], F32, isOutput=False)
    ind16_ext = nc.declare_dram_parameter("ind16", [128, 8], F32, isOutput=False)
    indT8_ext = nc.declare_dram_parameter("indT8", [8, 128], F32, isOutput=False)
    out_ext = nc.declare_dram_parameter("out", [C, NQ], F32, isOutput=True)

    with tile.TileContext(nc) as tc:
        _body(nc, tc, x_ext, xres_ext, w_ext, bias_ext, ind16_ext, indT8_ext, out_ext)
    _split_multi_waits(nc)
    return nc


def _body(nc, tc, x_ext, xres_ext, w_ext, bias_ext, ind16_ext, indT8_ext, out_ext):
    from contextlib import ExitStack

    ctx = ExitStack()
    with ctx:
        const = ctx.enter_context(tc.tile_pool(name="const", bufs=1))
        big = ctx.enter_context(tc.tile_pool(name="big", bufs=1))
        mm_psum = ctx.enter_context(tc.tile_pool(name="mm_psum", bufs=3, space="PSUM"))
        gn_pool = ctx.enter_context(tc.tile_pool(name="gn", bufs=4))

        # ---- x DMAs first, split across the SP hwdge queue and the Pool
        # software-DGE queue (measured concurrent at high aggregate BW) ----
        xf = [gn_pool.tile([128, N], F32, tag="xf", name=f"xf_{ct}") for ct in range(CT)]
        for ct in range(CT):
            eng = nc.sync if ct < 2 else nc.gpsimd
            eng.dma_start(out=xf[ct], in_=x_ext[ct * 128 : (ct + 1) * 128, :])

        # ---- constants + weights ride the Activation HWDGE queue ------
        ind16 = const.tile([128, 8], F32, tag="ind16")
        nc.scalar.dma_start(out=ind16, in_=ind16_ext[:])
        indT8 = const.tile([8, 128], F32, tag="indT8")
        nc.scalar.dma_start(out=indT8, in_=indT8_ext[:])
        biases = const.tile([128, 5 * CT], F32, tag="biases")
        nc.scalar.dma_start(out=biases, in_=bias_ext[:])
        bq_sb = biases[:, 0 * CT : 1 * CT]
        bk_sb = biases[:, 1 * CT : 2 * CT]
        bp_sb = biases[:, 2 * CT : 3 * CT]
        gnw_sb = biases[:, 3 * CT : 4 * CT]
        gnb_sb = biases[:, 4 * CT : 5 * CT]

        w8 = {}
        for k in ("wq8", "wk8", "wv8", "wp8"):
            wt = big.tile([128, CT, C], FP8, tag=f"w8_{k}")
            nc.scalar.dma_start(
                out=wt, in_=w_ext[k][:].rearrange("p (ct c) -> p ct c", ct=CT)
            )
            w8[k] = wt

        # DoubleRow lhsT needs the pair-dim step to be 16B-aligned, so pad
        # the ones column out to 16 and slice.
        ones_dr_full = const.tile([128, 2, 16], FP8, tag="ones_dr")
        nc.vector.memset(ones_dr_full, 1.0)
        ones_dr = ones_dr_full[:, :, 0:1]
        ones_row = const.tile([1, 128], BF16, tag="ones_row")
        nc.vector.memset(ones_row, 1.0)
        expshift = const.tile([128, 1], F32, tag="expshift")
        nc.vector.memset(expshift, EXP_SHIFT)

        # ---- persistent activations ------------------------------------
        t8 = big.tile([128, CT, N], FP8, tag="t8")
        xb = big.tile([128, CT, NQ], F32, tag="xb")
        q8 = big.tile([128, CT, NQ], FP8, tag="q8")
        k8 = big.tile([128, CT, N], FP8, tag="k8")
        v8 = big.tile([128, MT, C], FP8, tag="v8")
        # residual staging via DMA (queue FIFO puts it behind x0/x1 on SP;
        # it is only consumed by the first deferred projection ~120us in)
        nc.sync.dma_start(
            out=xb, in_=xres_ext[:].rearrange("(ct p) n -> p ct n", p=128)
        )

        # ---- phase 1: GroupNorm ----------------------------------------
        with (
            tc.tile_pool(name="gn_small", bufs=4) as small,
            tc.tile_pool(name="gn_psum", bufs=2, space="PSUM") as gn_psum,
        ):
            for ct in range(CT):
                x_t = xf[ct]
                xf3 = x_t.rearrange("p (c f) -> p c f", f=512)
                stats6 = small.tile([128, N // 512, 6], F32, tag="stats6")
                for c in range(N // 512):
                    nc.vector.bn_stats(out=stats6[:, c, :], in_=xf3[:, c, :])
                mv = small.tile([128, 2], F32, tag="mv")
                nc.vector.bn_aggr(out=mv, in_=stats6)

                # stats2 = [mean_c, E[x^2]_c]  (SBUF-only smalls ride Pool so
                # DVE stays on the bn_stats critical path)
                stats2 = small.tile([128, 2], F32, tag="stats2")
                nc.gpsimd.tensor_copy(out=stats2[:, 0:1], in_=mv[:, 0:1])
                nc.gpsimd.tensor_mul(stats2[:, 1:2], mv[:, 0:1], mv[:, 0:1])
                nc.gpsimd.tensor_add(stats2[:, 1:2], stats2[:, 1:2], mv[:, 1:2])

                # group aggregation: [8, 2] = (1/16) * sum over 16-ch groups
                gpsum = gn_psum.tile([8, 2], F32, tag="gpsum")
                nc.tensor.matmul(gpsum, lhsT=ind16, rhs=stats2, start=True, stop=True)

                gss = small.tile([8, 2], F32, tag="gss")
                nc.scalar.activation(gss, gpsum, IDENT)  # PSUM->SBUF (Pool can't)
                g_sb = small.tile([8, 2], F32, tag="g_sb")
                nc.gpsimd.tensor_copy(out=g_sb[:, 0:1], in_=gss[:, 0:1])
                msqg = small.tile([8, 1], F32, tag="msqg")
                nc.gpsimd.tensor_mul(msqg, gss[:, 0:1], gss[:, 0:1])
                epsm = small.tile([8, 1], F32, tag="epsm")
                nc.gpsimd.tensor_scalar(
                    epsm, msqg, -1.0, EPS, op0=MULT, op1=ADD,
                )
                stdg = small.tile([8, 1], F32, tag="stdg")
                nc.scalar.activation(stdg, gss[:, 1:2], SQRT, bias=epsm, scale=1.0)
                nc.vector.reciprocal(out=g_sb[:, 1:2], in_=stdg)

                # broadcast per-group -> per-channel: [128, 2] = indT8^T @ g_sb
                ppsum = gn_psum.tile([128, 2], F32, tag="ppsum")
                nc.tensor.matmul(ppsum, lhsT=indT8, rhs=g_sb, start=True, stop=True)

                # alpha/beta read ppsum (PSUM): ScalarE IDENT with AP scale
                alpha = small.tile([128, 1], F32, tag="alpha")
                nc.scalar.activation(
                    alpha, ppsum[:, 1:2], IDENT, scale=gnw_sb[:, ct : ct + 1]
                )
                beta = small.tile([128, 1], F32, tag="beta")
                nc.scalar.activation(beta, ppsum[:, 0:1], IDENT, scale=alpha)
                nc.gpsimd.tensor_sub(beta, gnb_sb[:, ct : ct + 1], beta)

                # t8 = alpha*x + beta, straight to fp8
                nc.scalar.activation(t8[:, ct, :], x_t, IDENT, bias=beta, scale=alpha)

        # ---- phase 2: Q / K / V projections (fp8 DoubleRow) ------------
        # evictions alternate DVE / ScalarE so neither trails the PE
        evict_ctr = [0]

        def evict(dst, src, bias_ap):
            if evict_ctr[0] % 2 == 0:
                if bias_ap is None:
                    nc.vector.tensor_scalar_mul(dst, src, 1.0 / WS)
                else:
                    nc.vector.tensor_scalar(dst, src, 1.0 / WS, bias_ap, op0=MULT, op1=ADD)
            else:
                nc.scalar.activation(
                    dst, src, IDENT,
                    bias=(0.0 if bias_ap is None else bias_ap), scale=1.0 / WS,
                )
            evict_ctr[0] += 1

        for dt in range(CT):
            for nb in range(QB):
                qp = mm_psum.tile([128, FB], F32, tag="mm")
                for i in range(CT // 2):
                    nc.tensor.matmul(
                        qp,
                        lhsT=w8["wq8"][:, 2 * i : 2 * i + 2, dt * 128 : (dt + 1) * 128],
                        rhs=t8[:, 2 * i : 2 * i + 2, nb * FB : (nb + 1) * FB],
                        start=(i == 0),
                        stop=(i == CT // 2 - 1),
                        perf_mode=DR,
                    )
                evict(q8[:, dt, nb * FB : (nb + 1) * FB], qp, bq_sb[:, dt : dt + 1])
        for dt in range(CT):
            for nb in range(NB):
                kp = mm_psum.tile([128, FB], F32, tag="mm")
                for i in range(CT // 2):
                    nc.tensor.matmul(
                        kp,
                        lhsT=w8["wk8"][:, 2 * i : 2 * i + 2, dt * 128 : (dt + 1) * 128],
                        rhs=t8[:, 2 * i : 2 * i + 2, nb * FB : (nb + 1) * FB],
                        start=(i == 0),
                        stop=(i == CT // 2 - 1),
                        perf_mode=DR,
                    )
                evict(k8[:, dt, nb * FB : (nb + 1) * FB], kp, bk_sb[:, dt : dt + 1])
        for mt in range(MT):
            vp = mm_psum.tile([128, C], F32, tag="mm")
            for i in range(CT // 2):
                nc.tensor.matmul(
                    vp,
                    lhsT=t8[:, 2 * i : 2 * i + 2, mt * 128 : (mt + 1) * 128],
                    rhs=w8["wv8"][:, 2 * i : 2 * i + 2, :],
                    start=(i == 0),
                    stop=(i == CT // 2 - 1),
                    perf_mode=DR,
                )
            evict(v8[:, mt, :], vp, None)  # bv folded into bp_eff on host

        # ---- phase 3: attention + proj + residual ----------------------
        with (
            tc.tile_pool(name="o_psum", bufs=1, space="PSUM") as o_psum,
            tc.tile_pool(name="lrb_psum", bufs=1, space="PSUM") as lrb_psum,
            tc.tile_pool(name="e_pool", bufs=4) as e_pool,
            tc.tile_pool(name="att_sb", bufs=2) as att_sb,
            tc.tile_pool(name="y_pool", bufs=4) as y_pool,
        ):
            prev = {}  # qb-1 state: o_sb, rb_sb, qs — projected during qb's S loop

            def deferred_proj(tail=False):
                if not prev:
                    return
                o_prev, rb_prev, qs_prev = prev["o"], prev["rb"], prev["qs"]
                for pt in range(CT):
                    pj = mm_psum.tile([128, FB], F32, tag="mm")
                    for i in range(CT // 2):
                        nc.tensor.matmul(
                            pj,
                            lhsT=w8["wp8"][:, 2 * i : 2 * i + 2, pt * 128 : (pt + 1) * 128],
                            rhs=o_prev[:, 2 * i : 2 * i + 2, :],
                            start=(i == 0),
                            stop=(i == CT // 2 - 1),
                            perf_mode=DR,
                        )
                    y_tile = y_pool.tile([128, FB], F32, tag="y")
                    nc.vector.tensor_mul(y_tile, pj, rb_prev)
                    # Pool's slow tensor_add is fine mid-stream (hidden under
                    # PE) but serializes the kernel tail — use DVE there
                    eng = nc.vector if tail else nc.gpsimd
                    eng.tensor_add(y_tile, y_tile, xb[:, pt, qs_prev])
                    nc.sync.dma_start(
                        out=out_ext[pt * 128 : (pt + 1) * 128, qs_prev], in_=y_tile
                    )
                prev.clear()

            for qb in range(QB):
                qs = slice(qb * FB, (qb + 1) * FB)
                op = [
                    o_psum.tile([128, FB], F32, tag=f"o{dc}", name=f"o_{qb}_{dc}")
                    for dc in range(CT)
                ]
                lp = lrb_psum.tile([128, FB], F32, tag="lrb", name=f"l_{qb}")
                lp1 = lp[0:1, :]

                for pr in range(MT // 2):  # pairs of key tiles
                    etp = e_pool.tile([128, 2, FB], FP8, tag="etp")
                    for half in range(2):
                        mt = 2 * pr + half
                        sp = mm_psum.tile([128, FB], F32, tag="mm")
                        for i in range(CT // 2):
                            nc.tensor.matmul(
                                sp,
                                lhsT=k8[:, 2 * i : 2 * i + 2, mt * 128 : (mt + 1) * 128],
                                rhs=q8[:, 2 * i : 2 * i + 2, qs],
                                start=(i == 0),
                                stop=(i == CT // 2 - 1),
                                perf_mode=DR,
                            )
                        nc.scalar.activation(
                            etp[:, half, :], sp, EXP, bias=expshift, scale=SCALE
                        )
                    nc.tensor.matmul(
                        lp1,
                        lhsT=ones_dr,
                        rhs=etp,
                        start=(pr == 0),
                        stop=(pr == MT // 2 - 1),
                        perf_mode=DR,
                    )
                    for dc in range(CT):
                        nc.tensor.matmul(
                            op[dc],
                            lhsT=v8[:, 2 * pr : 2 * pr + 2, dc * 128 : (dc + 1) * 128],
                            rhs=etp,
                            start=(pr == 0),
                            stop=(pr == MT // 2 - 1),
                            perf_mode=DR,
                        )
                    if pr == 2:
                        deferred_proj()  # project qb-1 while qb's S stream runs

                # rb chain: l_sb = (L/8)/4 in bf16 -> broadcast via ones
                # matmul -> reciprocal => rb = 32/true_L
                l_sb = att_sb.tile([1, FB], BF16, tag="l_sb")
                nc.vector.tensor_scalar_mul(l_sb, lp1, 0.25)
                rbp = lrb_psum.tile([128, FB], F32, tag="lrb", name=f"rb_{qb}")
                nc.tensor.matmul(rbp, lhsT=ones_row, rhs=l_sb, start=True, stop=True)
                rb_sb = att_sb.tile([128, FB], F32, tag="rb_sb")
                nc.vector.reciprocal(out=rb_sb, in_=rbp)

                # evict unnormalized O as fp8 (x 1/64) for the fp8 proj
                o_sb = att_sb.tile([128, CT, FB], FP8, tag="o_sb")
                for dc in range(CT):
                    nc.vector.tensor_scalar_mul(o_sb[:, dc, :], op[dc], 1.0 / 64.0)

                prev.update({"o": o_sb, "rb": rb_sb, "qs": qs})

            deferred_proj(tail=True)  # tail: last query block


_NC_CACHE = None


def _get_nc():
    global _NC_CACHE
    if _NC_CACHE is None:
        _NC_CACHE = _build_nc()
    return _NC_CACHE


def _make_indicators():
    p = np.arange(128)
    ind16 = np.zeros((128, 8), np.float32)
    ind16[p, p // GSIZE] = 1.0 / GSIZE
    indT8 = np.zeros((8, 128), np.float32)
    indT8[p // GSIZE, p] = 1.0
    return ind16, indT8


def _prep_common(inputs):
    """Host-side packing: fp8 weights in DoubleRow pair layout + biases."""
    f8 = ml_dtypes.float8_e4m3fn
    common = {}
    for name, key in (("wq8", "wq"), ("wk8", "wk"), ("wv8", "wv"), ("wp8", "wp")):
        w = np.asarray(inputs[key], np.float32)
        wT16 = np.ascontiguousarray(w.T) * WS                  # [c, d] * 16
        w8 = wT16.astype(f8)                                   # quantize
        # [c, d] -> [128, CT, d] -> [128, CT*d]  (partition = c % 128)
        common[name] = np.ascontiguousarray(
            w8.reshape(CT, 128, C).transpose(1, 0, 2).reshape(128, CT * C)
        )

    wp_q = common["wp8"].reshape(128, CT, C).transpose(1, 0, 2).reshape(C, C)
    wp_deq = wp_q.astype(np.float32) / WS                      # [c, d] = wp.T quantized
    bv = np.asarray(inputs["bv"], np.float64)
    bp_eff = (
        np.asarray(inputs["bp"], np.float64)
        + wp_deq.astype(np.float64).T @ bv                     # wp @ bv
    ).astype(np.float32)
    common["_bp_eff"] = bp_eff  # host-only: folded into the xres staging

    def tile_vec(v):
        return np.ascontiguousarray(np.asarray(v, np.float32).reshape(CT, 128).T)

    biases = np.concatenate(
        [
            tile_vec(inputs["bq"]),
            tile_vec(inputs["bk"]),
            tile_vec(bp_eff),
            tile_vec(inputs["gn_w"]),
            tile_vec(inputs["gn_b"]),
        ],
        axis=1,
    )
    common["biases"] = np.ascontiguousarray(biases)
    common["ind16"], common["indT8"] = _make_indicators()
    return common


def _build_in_maps(inputs):
    x = np.ascontiguousarray(np.asarray(inputs["x"], dtype=np.float32))
    assert x.shape == (B, C, H, W), x.shape
    xf = x.reshape(B, C, N)

    common = _prep_common(inputs)
    bp_eff = common.pop("_bp_eff")

    in_maps = []
    for core in range(8):
        b, h = core // 2, core % 2
        if h == 0:
            xc = xf[b]
        else:
            # rotate so this core's query tokens land in columns 0..NQ-1
            xc = np.concatenate([xf[b][:, NQ:], xf[b][:, :NQ]], axis=1)
        xres = xf[b][:, h * NQ : (h + 1) * NQ] + bp_eff[:, None]
        in_maps.append(
            {
                "x": np.ascontiguousarray(xc),
                "xres": np.ascontiguousarray(xres),
                **common,
            }
        )
    return in_maps


def kernel(**inputs) -> np.ndarray:
    in_maps = _build_in_maps(inputs)
    nc = _get_nc()
    res = run_bass_kernel_spmd(nc, in_maps, core_ids=list(range(8)))

    out = np.empty((B, C, N), np.float32)
    for core in range(8):
        b, h = core // 2, core % 2
        out[b][:, h * NQ : (h + 1) * NQ] = res.results[core]["out"]
    return out.reshape(B, C, H, W)


# revision 41
# speedup vs baseline: 1.0415x; 1.0415x over previous
"""AttentionBlock (GroupNorm + single-head self-attention + residual) on
8 TRN2 NeuronCores.

Sharding: data-parallel over batch (4 images) x 2-way sequence-parallel
over query tokens => 8 cores, zero collectives. Each core receives one
full image x[b] [C=512, N=4096] (token columns rotated so that its own
2048 query tokens sit in columns 0..2047), computes GroupNorm + K/V over
all 4096 tokens (K/V duplicated across the 2 cores of a batch pair --
cheaper than an all-gather at this size: the pair-exchange variant was
measured and the collective's ~35-45us latency sits on the critical
path, a net loss), Q / attention / proj / residual for its 2048
queries, and returns y [512, 2048].

All matmuls run fp8e4m3 with DoubleRow (K=256 per instruction, ~1.8x
bf16-equivalent throughput).  Weights are quantized to fp8 on the host
(x16 prescale so w*16 sits in fp8's sweet spot; the 1/16 is folded into
the PSUM evictions).  Host also folds wp@bv into an effective bp, so the
V eviction is a single scaled copy.

On-chip layout ("channels on partitions"):
  t8 = groupnorm(x)            [c, n]   fp8   (ScalarE activation)
  xb = x + bp_eff              [c, nq]  f32   (residual staging)
  Q8 = (wq16 @ t8)/16 + bq     [d, nq]  fp8
  K8 = (wk16 @ t8)/16 + bk     [d, m]   fp8
  V8 = (t8^T @ wv16T)/16       [m, d]   fp8   (computed directly transposed)
  S^T[m, nq] = K8^T Q8         (PE, 2 DR matmuls per 128-key tile)
  E  = exp(S^T*SCALE + shift)  (ScalarE, PSUM->SBUF, fp8; shift=-ln 8)
  L[nq]   = ones^T @ E         (PE accumulate, = true_L/8)
  Ou[d,nq] = V8^T @ E          (PE accumulate, = true_A/8)
  o8 = Ou/64 (fp8);  rb = 4/L broadcast (recip of ones x (L/4))
  y  = (wp16 @ o8) * rb + xb   -> DMA out   [proj scale: 16/512 * 32 = 1]

Softmax skips the max-subtraction: logits are ~N(0,1) by construction.
DMA order matters: the 8MB x load is issued first on the SP queue (it
fans out across DMA engines); constants/weights ride the Activation
HWDGE queue so nothing serializes ahead of x.
"""

import sys

for _p in ("/opt/trn_rl_repo", "/opt/pypackages"):
    if _p not in sys.path:
        sys.path.append(_p)

import ml_dtypes
import numpy as np

import concourse.bass as bass
import concourse.tile as tile
from concourse import mybir
from concourse.bass_utils import run_bass_kernel_spmd
from concourse.vector_clock import ScopedClock

# ----------------------------------------------------------------------
# Problem constants (nn_AttentionBlock_24764781429183)
B, C, H, W = 4, 512, 64, 64
N = H * W              # 4096 tokens
NQ = N // 2            # 2048 query tokens per core
GROUPS = 32
GSIZE = C // GROUPS    # 16 channels per group
EPS = 1e-5
SCALE = 1.0 / float(np.sqrt(C))
CT = C // 128          # 4 channel tiles
MT = N // 128          # 32 key tiles
FB = 512               # matmul free-dim block
QB = NQ // FB          # 4 query blocks
NB = N // FB           # 8 token blocks
WS = 16.0              # host-side weight prescale before fp8 quantization

F32 = mybir.dt.float32
BF16 = mybir.dt.bfloat16
FP8 = mybir.dt.float8e4
DR = mybir.MatmulPerfMode.DoubleRow
IDENT = mybir.ActivationFunctionType.Identity
EXP = mybir.ActivationFunctionType.Exp
SQRT = mybir.ActivationFunctionType.Sqrt
MULT = mybir.AluOpType.mult
ADD = mybir.AluOpType.add
# exp(s*SCALE + EXP_SHIFT) = exp(s*SCALE)/8 — keeps E safely inside
# fp8e4m3 range (max 448) even for outlier logits; cancels in E/L.
EXP_SHIFT = -2.0794415416798357


# ----------------------------------------------------------------------
# This container's walrus build rejects >1 semaphore wait on one CTRL
# (Drain) instruction; split the Tile end-of-kernel drain waits across
# one-nop-per-wait instead.
def _patched_drain_and_barrier(self, tick_clock, wait_clock):
    nc = self.nc
    probe = nc.sync.nop(nofuse=True)
    wait_clock.add_sem_waits(probe.ins, ScopedClock({None: tick_clock.global_clock}))
    sync_info = probe.ins.sync_info
    waits = list(sync_info.on_wait or []) if sync_info is not None else []
    if sync_info is not None and len(waits) > 1:
        sync_info.on_wait = waits[:1]
        for w in waits[1:]:
            n = nc.sync.nop(nofuse=True)
            if n.ins.sync_info is None:
                n.ins.sync_info = type(sync_info)(on_wait=[w], on_update=[])
            else:
                n.ins.sync_info.on_wait = [w]
    nc.sync.drain()
    nc.all_engine_barrier()
    assert self.sems is not None
    popped = nc._tile_sem_poison_stack.pop()
    assert popped is self._sem_poison
    nc.clear_and_free_semaphores(list(self.sems.allocated().values()))
    nc.all_engine_barrier()


tile.TileContext._drain_and_barrier = _patched_drain_and_barrier


# Disk-cache compiled NEFFs by BIR hash — the bass_exec compile path
# bypasses libneuronxla's HLO-keyed cache, so without this every fresh
# process pays the full (~6 min) walrus compile.
def _install_neff_cache():
    import hashlib
    import os
    import shutil

    import concourse.bass2jax as bass2jax

    if getattr(bass2jax, "_neff_cache_installed", False):
        return
    orig = bass2jax.compile_bir_kernel

    def cached(bir_json, tmpdir, neff_name="file.neff"):
        cdir = os.environ.get("BASS_NEFF_CACHE", "/tmp/bass_neff_cache")
        os.makedirs(cdir, exist_ok=True)
        cpath = os.path.join(cdir, hashlib.sha256(bir_json).hexdigest()[:32] + ".neff")
        dst = os.path.join(tmpdir, neff_name)
        if os.path.exists(cpath):
            shutil.copy(cpath, dst)
            return dst
        out = orig(bir_json, tmpdir, neff_name=neff_name)
        try:
            shutil.copy(out, cpath)
        except OSError:
            pass
        return out

    bass2jax.compile_bir_kernel = cached
    bass2jax._neff_cache_installed = True


_install_neff_cache()


def _split_multi_waits(nc: bass.Bass, maxw: int = 1) -> None:
    """Walrus in this container rejects instructions carrying more than one
    semaphore wait. Hoist extra waits onto same-engine no-ops inserted
    right before the instruction (engine streams execute in block order,
    so the waits still gate the instruction)."""
    ctr = 0
    for fn in nc.m.functions:
        for bb in fn.blocks:
            out = []
            changed = False
            for inst in bb.instructions:
                si = inst.sync_info
                waits = list(si.on_wait) if (si is not None and si.on_wait) else []
                if len(waits) > maxw and inst.engine != mybir.EngineType.Unassigned:
                    keep = waits[-maxw:]
                    for i in range(0, len(waits) - maxw, maxw):
                        nop = mybir.InstNoOp(name=f"waitsplit-{ctr}")
                        ctr += 1
                        nop.engine = inst.engine
                        nop.sync_info = mybir.SyncInfo(
                            on_wait=waits[i : i + maxw], on_update=[]
                        )
                        out.append(nop)
                    si.on_wait = keep
                    inst.sync_info = si
                    changed = True
                out.append(inst)
            if changed:
                bb.instructions = out


# ----------------------------------------------------------------------
def _build_nc() -> bass.Bass:
    nc = bass.Bass()

    x_ext = nc.declare_dram_parameter("x", [C, N], F32, isOutput=False)
    # residual staging x_own + bp_eff, DMA'd straight into SBUF (keeps the
    # per-channel bias add off ScalarE during the GroupNorm phase)
    xres_ext = nc.declare_dram_parameter("xres", [C, NQ], F32, isOutput=False)
    w_ext = {
        k: nc.declare_dram_parameter(k, [128, CT * C], FP8, isOutput=False)
        for k in ("wq8", "wk8", "wv8", "wp8")
    }
    # packed per-channel vectors: bq | bk | bp_eff | gnw | gnb, each [128, CT]
    bias_ext = nc.declare_dram_parameter("biases", [128, 5 * CT], F32, isOutput=False)
    ind16_ext = nc.declare_dram_parameter("ind16", [128, 8], F32, isOutput=False)
    indT8_ext = nc.declare_dram_parameter("indT8", [8, 128], F32, isOutput=False)
    out_ext = nc.declare_dram_parameter("out", [C, NQ], F32, isOutput=True)

    with tile.TileContext(nc) as tc:
        _body(nc, tc, x_ext, xres_ext, w_ext, bias_ext, ind16_ext, indT8_ext, out_ext)
    _split_multi_waits(nc)
    return nc


def _body(nc, tc, x_ext, xres_ext, w_ext, bias_ext, ind16_ext, indT8_ext, out_ext):
    from contextlib import ExitStack

    ctx = ExitStack()
    with ctx:
        const = ctx.enter_context(tc.tile_pool(name="const", bufs=1))
        big = ctx.enter_context(tc.tile_pool(name="big", bufs=1))
        mm_psum = ctx.enter_context(tc.tile_pool(name="mm_psum", bufs=3, space="PSUM"))
        gn_pool = ctx.enter_context(tc.tile_pool(name="gn", bufs=4))

        # ---- x DMAs first: 8MB on the SP queue, descriptors fan out.
        # (Splitting across queues was measured slower: concurrent queues
        # steal bandwidth from the first tile, delaying the stats start.)
        xf = [gn_pool.tile([128, N], F32, tag="xf", name=f"xf_{ct}") for ct in range(CT)]
        for ct in range(CT):
            nc.sync.dma_start(out=xf[ct], in_=x_ext[ct * 128 : (ct + 1) * 128, :])

        # ---- constants + weights ride the Activation HWDGE queue ------
        ind16 = const.tile([128, 8], F32, tag="ind16")
        nc.scalar.dma_start(out=ind16, in_=ind16_ext[:])
        indT8 = const.tile([8, 128], F32, tag="indT8")
        nc.scalar.dma_start(out=indT8, in_=indT8_ext[:])
        biases = const.tile([128, 5 * CT], F32, tag="biases")
        nc.scalar.dma_start(out=biases, in_=bias_ext[:])
        bq_sb = biases[:, 0 * CT : 1 * CT]
        bk_sb = biases[:, 1 * CT : 2 * CT]
        bp_sb = biases[:, 2 * CT : 3 * CT]
        gnw_sb = biases[:, 3 * CT : 4 * CT]
        gnb_sb = biases[:, 4 * CT : 5 * CT]

        w8 = {}
        for k in ("wq8", "wk8", "wv8", "wp8"):
            wt = big.tile([128, CT, C], FP8, tag=f"w8_{k}")
            nc.scalar.dma_start(
                out=wt, in_=w_ext[k][:].rearrange("p (ct c) -> p ct c", ct=CT)
            )
            w8[k] = wt

        # DoubleRow lhsT needs the pair-dim step to be 16B-aligned, so pad
        # the ones column out to 16 and slice.
        ones_dr_full = const.tile([128, 2, 16], FP8, tag="ones_dr")
        nc.vector.memset(ones_dr_full, 1.0)
        ones_dr = ones_dr_full[:, :, 0:1]
        ones_row = const.tile([1, 128], BF16, tag="ones_row")
        nc.vector.memset(ones_row, 1.0)
        expshift = const.tile([128, 1], F32, tag="expshift")
        nc.vector.memset(expshift, EXP_SHIFT)

        # ---- persistent activations ------------------------------------
        t8 = big.tile([128, CT, N], FP8, tag="t8")
        xb = big.tile([128, CT, NQ], F32, tag="xb")
        q8 = big.tile([128, CT, NQ], FP8, tag="q8")
        k8 = big.tile([128, CT, N], FP8, tag="k8")
        v8 = big.tile([128, MT, C], FP8, tag="v8")
        # residual staging via DMA (queue FIFO puts it behind x0/x1 on SP;
        # it is only consumed by the first deferred projection ~120us in)
        nc.sync.dma_start(
            out=xb, in_=xres_ext[:].rearrange("(ct p) n -> p ct n", p=128)
        )

        # ---- phase 1: GroupNorm ----------------------------------------
        with (
            tc.tile_pool(name="gn_small", bufs=4) as small,
            tc.tile_pool(name="gn_psum", bufs=2, space="PSUM") as gn_psum,
        ):
            for ct in range(CT):
                x_t = xf[ct]
                xf3 = x_t.rearrange("p (c f) -> p c f", f=512)
                stats6 = small.tile([128, N // 512, 6], F32, tag="stats6")
                for c in range(N // 512):
                    nc.vector.bn_stats(out=stats6[:, c, :], in_=xf3[:, c, :])
                mv = small.tile([128, 2], F32, tag="mv")
                nc.vector.bn_aggr(out=mv, in_=stats6)

                # stats2 = [mean_c, E[x^2]_c]  (SBUF-only smalls ride Pool so
                # DVE stays on the bn_stats critical path)
                stats2 = small.tile([128, 2], F32, tag="stats2")
                nc.gpsimd.tensor_copy(out=stats2[:, 0:1], in_=mv[:, 0:1])
                nc.gpsimd.tensor_mul(stats2[:, 1:2], mv[:, 0:1], mv[:, 0:1])
                nc.gpsimd.tensor_add(stats2[:, 1:2], stats2[:, 1:2], mv[:, 1:2])

                # group aggregation: [8, 2] = (1/16) * sum over 16-ch groups
                gpsum = gn_psum.tile([8, 2], F32, tag="gpsum")
                nc.tensor.matmul(gpsum, lhsT=ind16, rhs=stats2, start=True, stop=True)

                gss = small.tile([8, 2], F32, tag="gss")
                nc.scalar.activation(gss, gpsum, IDENT)  # PSUM->SBUF (Pool can't)
                g_sb = small.tile([8, 2], F32, tag="g_sb")
                nc.gpsimd.tensor_copy(out=g_sb[:, 0:1], in_=gss[:, 0:1])
                msqg = small.tile([8, 1], F32, tag="msqg")
                nc.gpsimd.tensor_mul(msqg, gss[:, 0:1], gss[:, 0:1])
                epsm = small.tile([8, 1], F32, tag="epsm")
                nc.gpsimd.tensor_scalar(
                    epsm, msqg, -1.0, EPS, op0=MULT, op1=ADD,
                )
                stdg = small.tile([8, 1], F32, tag="stdg")
                nc.scalar.activation(stdg, gss[:, 1:2], SQRT, bias=epsm, scale=1.0)
                nc.vector.reciprocal(out=g_sb[:, 1:2], in_=stdg)

                # broadcast per-group -> per-channel: [128, 2] = indT8^T @ g_sb
                ppsum = gn_psum.tile([128, 2], F32, tag="ppsum")
                nc.tensor.matmul(ppsum, lhsT=indT8, rhs=g_sb, start=True, stop=True)

                # alpha/beta read ppsum (PSUM): ScalarE IDENT with AP scale
                alpha = small.tile([128, 1], F32, tag="alpha")
                nc.scalar.activation(
                    alpha, ppsum[:, 1:2], IDENT, scale=gnw_sb[:, ct : ct + 1]
                )
                beta = small.tile([128, 1], F32, tag="beta")
                nc.scalar.activation(beta, ppsum[:, 0:1], IDENT, scale=alpha)
                nc.gpsimd.tensor_sub(beta, gnb_sb[:, ct : ct + 1], beta)

                # t8 = alpha*x + beta, straight to fp8
                nc.scalar.activation(t8[:, ct, :], x_t, IDENT, bias=beta, scale=alpha)

        # ---- phase 2: Q / K / V projections (fp8 DoubleRow) ------------
        # evictions alternate DVE / ScalarE so neither trails the PE
        evict_ctr = [0]

        def evict(dst, src, bias_ap):
            if evict_ctr[0] % 2 == 0:
                if bias_ap is None:
                    nc.vector.tensor_scalar_mul(dst, src, 1.0 / WS)
                else:
                    nc.vector.tensor_scalar(dst, src, 1.0 / WS, bias_ap, op0=MULT, op1=ADD)
            else:
                nc.scalar.activation(
                    dst, src, IDENT,
                    bias=(0.0 if bias_ap is None else bias_ap), scale=1.0 / WS,
                )
            evict_ctr[0] += 1

        for dt in range(CT):
            for nb in range(QB):
                qp = mm_psum.tile([128, FB], F32, tag="mm")
                for i in range(CT // 2):
                    nc.tensor.matmul(
                        qp,
                        lhsT=w8["wq8"][:, 2 * i : 2 * i + 2, dt * 128 : (dt + 1) * 128],
                        rhs=t8[:, 2 * i : 2 * i + 2, nb * FB : (nb + 1) * FB],
                        start=(i == 0),
                        stop=(i == CT // 2 - 1),
                        perf_mode=DR,
                    )
                evict(q8[:, dt, nb * FB : (nb + 1) * FB], qp, bq_sb[:, dt : dt + 1])
        for dt in range(CT):
            for nb in range(NB):
                kp = mm_psum.tile([128, FB], F32, tag="mm")
                for i in range(CT // 2):
                    nc.tensor.matmul(
                        kp,
                        lhsT=w8["wk8"][:, 2 * i : 2 * i + 2, dt * 128 : (dt + 1) * 128],
                        rhs=t8[:, 2 * i : 2 * i + 2, nb * FB : (nb + 1) * FB],
                        start=(i == 0),
                        stop=(i == CT // 2 - 1),
                        perf_mode=DR,
                    )
                evict(k8[:, dt, nb * FB : (nb + 1) * FB], kp, bk_sb[:, dt : dt + 1])
        for mt in range(MT):
            vp = mm_psum.tile([128, C], F32, tag="mm")
            for i in range(CT // 2):
                nc.tensor.matmul(
                    vp,
                    lhsT=t8[:, 2 * i : 2 * i + 2, mt * 128 : (mt + 1) * 128],
                    rhs=w8["wv8"][:, 2 * i : 2 * i + 2, :],
                    start=(i == 0),
                    stop=(i == CT // 2 - 1),
                    perf_mode=DR,
                )
            evict(v8[:, mt, :], vp, None)  # bv folded into bp_eff on host

        # ---- phase 3: attention + proj + residual ----------------------
        with (
            tc.tile_pool(name="o_psum", bufs=1, space="PSUM") as o_psum,
            tc.tile_pool(name="lrb_psum", bufs=1, space="PSUM") as lrb_psum,
            tc.tile_pool(name="e_pool", bufs=4) as e_pool,
            tc.tile_pool(name="att_sb", bufs=2) as att_sb,
            tc.tile_pool(name="y_pool", bufs=4) as y_pool,
        ):
            prev = {}  # qb-1 state: o_sb, rb_sb, qs — projected during qb's S loop

            def deferred_proj(tail=False):
                if not prev:
                    return
                o_prev, rb_prev, qs_prev = prev["o"], prev["rb"], prev["qs"]
                for pt in range(CT):
                    pj = mm_psum.tile([128, FB], F32, tag="mm")
                    for i in range(CT // 2):
                        nc.tensor.matmul(
                            pj,
                            lhsT=w8["wp8"][:, 2 * i : 2 * i + 2, pt * 128 : (pt + 1) * 128],
                            rhs=o_prev[:, 2 * i : 2 * i + 2, :],
                            start=(i == 0),
                            stop=(i == CT // 2 - 1),
                            perf_mode=DR,
                        )
                    y_tile = y_pool.tile([128, FB], F32, tag="y")
                    nc.vector.tensor_mul(y_tile, pj, rb_prev)
                    # Pool's slow tensor_add is fine mid-stream (hidden under
                    # PE) but serializes the kernel tail — use DVE there
                    eng = nc.vector if tail else nc.gpsimd
                    eng.tensor_add(y_tile, y_tile, xb[:, pt, qs_prev])
                    nc.sync.dma_start(
                        out=out_ext[pt * 128 : (pt + 1) * 128, qs_prev], in_=y_tile
                    )
                prev.clear()

            for qb in range(QB):
                qs = slice(qb * FB, (qb + 1) * FB)
                op = [
                    o_psum.tile([128, FB], F32, tag=f"o{dc}", name=f"o_{qb}_{dc}")
                    for dc in range(CT)
                ]
                lp = lrb_psum.tile([128, FB], F32, tag="lrb", name=f"l_{qb}")
                lp1 = lp[0:1, :]

                for pr in range(MT // 2):  # pairs of key tiles
                    etp = e_pool.tile([128, 2, FB], FP8, tag="etp")
                    for half in range(2):
                        mt = 2 * pr + half
                        sp = mm_psum.tile([128, FB], F32, tag="mm")
                        for i in range(CT // 2):
                            nc.tensor.matmul(
                                sp,
                                lhsT=k8[:, 2 * i : 2 * i + 2, mt * 128 : (mt + 1) * 128],
                                rhs=q8[:, 2 * i : 2 * i + 2, qs],
                                start=(i == 0),
                                stop=(i == CT // 2 - 1),
                                perf_mode=DR,
                            )
                        nc.scalar.activation(
                            etp[:, half, :], sp, EXP, bias=expshift, scale=SCALE
                        )
                    nc.tensor.matmul(
                        lp1,
                        lhsT=ones_dr,
                        rhs=etp,
                        start=(pr == 0),
                        stop=(pr == MT // 2 - 1),
                        perf_mode=DR,
                    )
                    for dc in range(CT):
                        nc.tensor.matmul(
                            op[dc],
                            lhsT=v8[:, 2 * pr : 2 * pr + 2, dc * 128 : (dc + 1) * 128],
                            rhs=etp,
                            start=(pr == 0),
                            stop=(pr == MT // 2 - 1),
                            perf_mode=DR,
                        )
                    if pr == 2:
                        deferred_proj()  # project qb-1 while qb's S stream runs

                # rb chain: l_sb = (L/8)/4 in bf16 -> broadcast via ones
                # matmul -> reciprocal => rb = 32/true_L
                l_sb = att_sb.tile([1, FB], BF16, tag="l_sb")
                nc.vector.tensor_scalar_mul(l_sb, lp1, 0.25)
                rbp = lrb_psum.tile([128, FB], F32, tag="lrb", name=f"rb_{qb}")
                nc.tensor.matmul(rbp, lhsT=ones_row, rhs=l_sb, start=True, stop=True)
                rb_sb = att_sb.tile([128, FB], F32, tag="rb_sb")
                nc.vector.reciprocal(out=rb_sb, in_=rbp)

                # evict unnormalized O as fp8 (x 1/64) for the fp8 proj
                o_sb = att_sb.tile([128, CT, FB], FP8, tag="o_sb")
                for dc in range(CT):
                    nc.vector.tensor_scalar_mul(o_sb[:, dc, :], op[dc], 1.0 / 64.0)

                prev.update({"o": o_sb, "rb": rb_sb, "qs": qs})

            deferred_proj(tail=True)  # tail: last query block


_NC_CACHE = None


def _get_nc():
    global _NC_CACHE
    if _NC_CACHE is None:
        _NC_CACHE = _build_nc()
    return _NC_CACHE


def _make_indicators():
    p = np.arange(128)
    ind16 = np.zeros((128, 8), np.float32)
    ind16[p, p // GSIZE] = 1.0 / GSIZE
    indT8 = np.zeros((8, 128), np.float32)
    indT8[p // GSIZE, p] = 1.0
    return ind16, indT8


def _prep_common(inputs):
    """Host-side packing: fp8 weights in DoubleRow pair layout + biases."""
    f8 = ml_dtypes.float8_e4m3fn
    common = {}
    for name, key in (("wq8", "wq"), ("wk8", "wk"), ("wv8", "wv"), ("wp8", "wp")):
        w = np.asarray(inputs[key], np.float32)
        wT16 = np.ascontiguousarray(w.T) * WS                  # [c, d] * 16
        w8 = wT16.astype(f8)                                   # quantize
        # [c, d] -> [128, CT, d] -> [128, CT*d]  (partition = c % 128)
        common[name] = np.ascontiguousarray(
            w8.reshape(CT, 128, C).transpose(1, 0, 2).reshape(128, CT * C)
        )

    wp_q = common["wp8"].reshape(128, CT, C).transpose(1, 0, 2).reshape(C, C)
    wp_deq = wp_q.astype(np.float32) / WS                      # [c, d] = wp.T quantized
    bv = np.asarray(inputs["bv"], np.float64)
    bp_eff = (
        np.asarray(inputs["bp"], np.float64)
        + wp_deq.astype(np.float64).T @ bv                     # wp @ bv
    ).astype(np.float32)
    common["_bp_eff"] = bp_eff  # host-only: folded into the xres staging

    def tile_vec(v):
        return np.ascontiguousarray(np.asarray(v, np.float32).reshape(CT, 128).T)

    biases = np.concatenate(
        [
            tile_vec(inputs["bq"]),
            tile_vec(inputs["bk"]),
            tile_vec(bp_eff),
            tile_vec(inputs["gn_w"]),
            tile_vec(inputs["gn_b"]),
        ],
        axis=1,
    )
    common["biases"] = np.ascontiguousarray(biases)
    common["ind16"], common["indT8"] = _make_indicators()
    return common


def _build_in_maps(inputs):
    x = np.ascontiguousarray(np.asarray(inputs["x"], dtype=np.float32))
    assert x.shape == (B, C, H, W), x.shape
    xf = x.reshape(B, C, N)

    common = _prep_common(inputs)
    bp_eff = common.pop("_bp_eff")

    in_maps = []
    for core in range(8):
        b, h = core // 2, core % 2
        if h == 0:
            xc = xf[b]
        else:
            # rotate so this core's query tokens land in columns 0..NQ-1
            xc = np.concatenate([xf[b][:, NQ:], xf[b][:, :NQ]], axis=1)
        xres = xf[b][:, h * NQ : (h + 1) * NQ] + bp_eff[:, None]
        in_maps.append(
            {
                "x": np.ascontiguousarray(xc),
                "xres": np.ascontiguousarray(xres),
                **common,
            }
        )
    return in_maps


def kernel(**inputs) -> np.ndarray:
    in_maps = _build_in_maps(inputs)
    nc = _get_nc()
    res = run_bass_kernel_spmd(nc, in_maps, core_ids=list(range(8)))

    out = np.empty((B, C, N), np.float32)
    for core in range(8):
        b, h = core // 2, core % 2
        out[b][:, h * NQ : (h + 1) * NQ] = res.results[core]["out"]
    return out.reshape(B, C, H, W)


# revision 44
# speedup vs baseline: 1.0586x; 1.0164x over previous
"""AttentionBlock (GroupNorm + single-head self-attention + residual) on
8 TRN2 NeuronCores.

Sharding: data-parallel over batch (4 images) x 2-way sequence-parallel
over query tokens => 8 cores, zero collectives. Each core receives one
full image x[b] [C=512, N=4096] (token columns rotated so that its own
2048 query tokens sit in columns 0..2047), computes GroupNorm + K/V over
all 4096 tokens (K/V duplicated across the 2 cores of a batch pair --
cheaper than an all-gather at this size: the pair-exchange variant was
measured and the collective's ~35-45us latency sits on the critical
path, a net loss), Q / attention / proj / residual for its 2048
queries, and returns y [512, 2048].

All matmuls run fp8e4m3 with DoubleRow (K=256 per instruction, ~1.8x
bf16-equivalent throughput).  Weights are quantized to fp8 on the host
(x16 prescale so w*16 sits in fp8's sweet spot; the 1/16 is folded into
the PSUM evictions).  Host also folds wp@bv into an effective bp, so the
V eviction is a single scaled copy.

On-chip layout ("channels on partitions"):
  t8 = groupnorm(x)            [c, n]   fp8   (ScalarE activation)
  xb = x + bp_eff              [c, nq]  f32   (residual staging)
  Q8 = (wq16 @ t8)/16 + bq     [d, nq]  fp8
  K8 = (wk16 @ t8)/16 + bk     [d, m]   fp8
  V8 = (t8^T @ wv16T)/16       [m, d]   fp8   (computed directly transposed)
  S^T[m, nq] = K8^T Q8         (PE, 2 DR matmuls per 128-key tile)
  E  = exp(S^T*SCALE + shift)  (ScalarE, PSUM->SBUF, fp8; shift=-ln 8)
  L[nq]   = ones^T @ E         (PE accumulate, = true_L/8)
  Ou[d,nq] = V8^T @ E          (PE accumulate, = true_A/8)
  o8 = Ou/64 (fp8);  rb = 4/L broadcast (recip of ones x (L/4))
  y  = (wp16 @ o8) * rb + xb   -> DMA out   [proj scale: 16/512 * 32 = 1]

Softmax skips the max-subtraction: logits are ~N(0,1) by construction.
DMA order matters: the 8MB x load is issued first on the SP queue (it
fans out across DMA engines); constants/weights ride the Activation
HWDGE queue so nothing serializes ahead of x.
"""

import sys

for _p in ("/opt/trn_rl_repo", "/opt/pypackages"):
    if _p not in sys.path:
        sys.path.append(_p)

import ml_dtypes
import numpy as np

import concourse.bass as bass
import concourse.tile as tile
from concourse import mybir
from concourse.bass_utils import run_bass_kernel_spmd
from concourse.vector_clock import ScopedClock

# ----------------------------------------------------------------------
# Problem constants (nn_AttentionBlock_24764781429183)
B, C, H, W = 4, 512, 64, 64
N = H * W              # 4096 tokens
NQ = N // 2            # 2048 query tokens per core
GROUPS = 32
GSIZE = C // GROUPS    # 16 channels per group
EPS = 1e-5
SCALE = 1.0 / float(np.sqrt(C))
CT = C // 128          # 4 channel tiles
MT = N // 128          # 32 key tiles
FB = 512               # matmul free-dim block
QB = NQ // FB          # 4 query blocks
NB = N // FB           # 8 token blocks
WS = 16.0              # host-side weight prescale before fp8 quantization

F32 = mybir.dt.float32
BF16 = mybir.dt.bfloat16
FP8 = mybir.dt.float8e4
DR = mybir.MatmulPerfMode.DoubleRow
IDENT = mybir.ActivationFunctionType.Identity
EXP = mybir.ActivationFunctionType.Exp
SQRT = mybir.ActivationFunctionType.Sqrt
MULT = mybir.AluOpType.mult
ADD = mybir.AluOpType.add
# exp(s*SCALE + EXP_SHIFT) = exp(s*SCALE)/8 — keeps E safely inside
# fp8e4m3 range (max 448) even for outlier logits; cancels in E/L.
EXP_SHIFT = -2.0794415416798357


# ----------------------------------------------------------------------
# This container's walrus build rejects >1 semaphore wait on one CTRL
# (Drain) instruction; split the Tile end-of-kernel drain waits across
# one-nop-per-wait instead.
def _patched_drain_and_barrier(self, tick_clock, wait_clock):
    nc = self.nc
    probe = nc.sync.nop(nofuse=True)
    wait_clock.add_sem_waits(probe.ins, ScopedClock({None: tick_clock.global_clock}))
    sync_info = probe.ins.sync_info
    waits = list(sync_info.on_wait or []) if sync_info is not None else []
    if sync_info is not None and len(waits) > 1:
        sync_info.on_wait = waits[:1]
        for w in waits[1:]:
            n = nc.sync.nop(nofuse=True)
            if n.ins.sync_info is None:
                n.ins.sync_info = type(sync_info)(on_wait=[w], on_update=[])
            else:
                n.ins.sync_info.on_wait = [w]
    nc.sync.drain()
    nc.all_engine_barrier()
    assert self.sems is not None
    popped = nc._tile_sem_poison_stack.pop()
    assert popped is self._sem_poison
    nc.clear_and_free_semaphores(list(self.sems.allocated().values()))
    nc.all_engine_barrier()


tile.TileContext._drain_and_barrier = _patched_drain_and_barrier


# Disk-cache compiled NEFFs by BIR hash — the bass_exec compile path
# bypasses libneuronxla's HLO-keyed cache, so without this every fresh
# process pays the full (~6 min) walrus compile.
def _install_neff_cache():
    import hashlib
    import os
    import shutil

    import concourse.bass2jax as bass2jax

    if getattr(bass2jax, "_neff_cache_installed", False):
        return
    orig = bass2jax.compile_bir_kernel

    def cached(bir_json, tmpdir, neff_name="file.neff"):
        cdir = os.environ.get("BASS_NEFF_CACHE", "/tmp/bass_neff_cache")
        os.makedirs(cdir, exist_ok=True)
        cpath = os.path.join(cdir, hashlib.sha256(bir_json).hexdigest()[:32] + ".neff")
        dst = os.path.join(tmpdir, neff_name)
        if os.path.exists(cpath):
            shutil.copy(cpath, dst)
            return dst
        out = orig(bir_json, tmpdir, neff_name=neff_name)
        try:
            shutil.copy(out, cpath)
        except OSError:
            pass
        return out

    bass2jax.compile_bir_kernel = cached
    bass2jax._neff_cache_installed = True


_install_neff_cache()


def _split_multi_waits(nc: bass.Bass, maxw: int = 1) -> None:
    """Walrus in this container rejects instructions carrying more than one
    semaphore wait. Hoist extra waits onto same-engine no-ops inserted
    right before the instruction (engine streams execute in block order,
    so the waits still gate the instruction)."""
    ctr = 0
    for fn in nc.m.functions:
        for bb in fn.blocks:
            out = []
            changed = False
            for inst in bb.instructions:
                si = inst.sync_info
                waits = list(si.on_wait) if (si is not None and si.on_wait) else []
                if len(waits) > maxw and inst.engine != mybir.EngineType.Unassigned:
                    keep = waits[-maxw:]
                    for i in range(0, len(waits) - maxw, maxw):
                        nop = mybir.InstNoOp(name=f"waitsplit-{ctr}")
                        ctr += 1
                        nop.engine = inst.engine
                        nop.sync_info = mybir.SyncInfo(
                            on_wait=waits[i : i + maxw], on_update=[]
                        )
                        out.append(nop)
                    si.on_wait = keep
                    inst.sync_info = si
                    changed = True
                out.append(inst)
            if changed:
                bb.instructions = out


# ----------------------------------------------------------------------
def _build_nc() -> bass.Bass:
    nc = bass.Bass()

    x_ext = nc.declare_dram_parameter("x", [C, N], F32, isOutput=False)
    # residual staging x_own + bp_eff, DMA'd straight into SBUF (keeps the
    # per-channel bias add off ScalarE during the GroupNorm phase)
    xres_ext = nc.declare_dram_parameter("xres", [C, NQ], F32, isOutput=False)
    w_ext = {
        k: nc.declare_dram_parameter(k, [128, CT * C], FP8, isOutput=False)
        for k in ("wq8", "wk8", "wv8", "wp8")
    }
    # packed per-channel vectors: bq | bk | bp_eff | gnw | gnb, each [128, CT]
    bias_ext = nc.declare_dram_parameter("biases", [128, 5 * CT], F32, isOutput=False)
    ind16_ext = nc.declare_dram_parameter("ind16", [128, 8], F32, isOutput=False)
    indT8_ext = nc.declare_dram_parameter("indT8", [8, 128], F32, isOutput=False)
    out_ext = nc.declare_dram_parameter("out", [C, NQ], F32, isOutput=True)

    with tile.TileContext(nc) as tc:
        _body(nc, tc, x_ext, xres_ext, w_ext, bias_ext, ind16_ext, indT8_ext, out_ext)
    _split_multi_waits(nc)
    return nc


def _body(nc, tc, x_ext, xres_ext, w_ext, bias_ext, ind16_ext, indT8_ext, out_ext):
    from contextlib import ExitStack

    ctx = ExitStack()
    with ctx:
        const = ctx.enter_context(tc.tile_pool(name="const", bufs=1))
        big = ctx.enter_context(tc.tile_pool(name="big", bufs=1))
        mm_psum = ctx.enter_context(tc.tile_pool(name="mm_psum", bufs=3, space="PSUM"))
        gn_pool = ctx.enter_context(tc.tile_pool(name="gn", bufs=4))

        # ---- x DMAs first: 8MB on the SP queue, descriptors fan out.
        # (Splitting across queues was measured slower: concurrent queues
        # steal bandwidth from the first tile, delaying the stats start.)
        xf = [gn_pool.tile([128, N], F32, tag="xf", name=f"xf_{ct}") for ct in range(CT)]
        for ct in range(CT):
            nc.sync.dma_start(out=xf[ct], in_=x_ext[ct * 128 : (ct + 1) * 128, :])

        # ---- constants + weights ride the Activation HWDGE queue ------
        ind16 = const.tile([128, 8], F32, tag="ind16")
        nc.scalar.dma_start(out=ind16, in_=ind16_ext[:])
        indT8 = const.tile([8, 128], F32, tag="indT8")
        nc.scalar.dma_start(out=indT8, in_=indT8_ext[:])
        biases = const.tile([128, 5 * CT], F32, tag="biases")
        nc.scalar.dma_start(out=biases, in_=bias_ext[:])
        bq_sb = biases[:, 0 * CT : 1 * CT]
        bk_sb = biases[:, 1 * CT : 2 * CT]
        bp_sb = biases[:, 2 * CT : 3 * CT]
        gnw_sb = biases[:, 3 * CT : 4 * CT]
        gnb_sb = biases[:, 4 * CT : 5 * CT]

        w8 = {}
        for k in ("wq8", "wk8", "wv8", "wp8"):
            wt = big.tile([128, CT, C], FP8, tag=f"w8_{k}")
            nc.scalar.dma_start(
                out=wt, in_=w_ext[k][:].rearrange("p (ct c) -> p ct c", ct=CT)
            )
            w8[k] = wt

        # DoubleRow lhsT needs the pair-dim step to be 16B-aligned, so pad
        # the ones column out to 16 and slice.
        ones_dr_full = const.tile([128, 2, 16], FP8, tag="ones_dr")
        nc.vector.memset(ones_dr_full, 1.0)
        ones_dr = ones_dr_full[:, :, 0:1]
        ones_row = const.tile([1, 128], BF16, tag="ones_row")
        nc.vector.memset(ones_row, 1.0)
        expshift = const.tile([128, 1], F32, tag="expshift")
        nc.vector.memset(expshift, EXP_SHIFT)

        # ---- persistent activations ------------------------------------
        t8 = big.tile([128, CT, N], FP8, tag="t8")
        xb = big.tile([128, CT, NQ], F32, tag="xb")
        q8 = big.tile([128, CT, NQ], FP8, tag="q8")
        k8 = big.tile([128, CT, N], FP8, tag="k8")
        v8 = big.tile([128, MT, C], FP8, tag="v8")
        # residual staging via DMA (queue FIFO puts it behind x0/x1 on SP;
        # it is only consumed by the first deferred projection ~120us in)
        nc.sync.dma_start(
            out=xb, in_=xres_ext[:].rearrange("(ct p) n -> p ct n", p=128)
        )

        # ---- phase 1: GroupNorm ----------------------------------------
        with (
            tc.tile_pool(name="gn_small", bufs=4) as small,
            tc.tile_pool(name="gn_psum", bufs=2, space="PSUM") as gn_psum,
        ):
            for ct in range(CT):
                x_t = xf[ct]
                xf3 = x_t.rearrange("p (c f) -> p c f", f=512)
                stats6 = small.tile([128, N // 512, 6], F32, tag="stats6")
                for c in range(N // 512):
                    nc.vector.bn_stats(out=stats6[:, c, :], in_=xf3[:, c, :])
                mv = small.tile([128, 2], F32, tag="mv")
                nc.vector.bn_aggr(out=mv, in_=stats6)

                # stats2 = [mean_c, E[x^2]_c]  (SBUF-only smalls ride Pool so
                # DVE stays on the bn_stats critical path)
                stats2 = small.tile([128, 2], F32, tag="stats2")
                nc.gpsimd.tensor_copy(out=stats2[:, 0:1], in_=mv[:, 0:1])
                nc.gpsimd.tensor_mul(stats2[:, 1:2], mv[:, 0:1], mv[:, 0:1])
                nc.gpsimd.tensor_add(stats2[:, 1:2], stats2[:, 1:2], mv[:, 1:2])

                # group aggregation: [8, 2] = (1/16) * sum over 16-ch groups
                gpsum = gn_psum.tile([8, 2], F32, tag="gpsum")
                nc.tensor.matmul(gpsum, lhsT=ind16, rhs=stats2, start=True, stop=True)

                gss = small.tile([8, 2], F32, tag="gss")
                nc.scalar.activation(gss, gpsum, IDENT)  # PSUM->SBUF (Pool can't)
                g_sb = small.tile([8, 2], F32, tag="g_sb")
                nc.gpsimd.tensor_copy(out=g_sb[:, 0:1], in_=gss[:, 0:1])
                msqg = small.tile([8, 1], F32, tag="msqg")
                nc.gpsimd.tensor_mul(msqg, gss[:, 0:1], gss[:, 0:1])
                epsm = small.tile([8, 1], F32, tag="epsm")
                nc.gpsimd.tensor_scalar(
                    epsm, msqg, -1.0, EPS, op0=MULT, op1=ADD,
                )
                stdg = small.tile([8, 1], F32, tag="stdg")
                nc.scalar.activation(stdg, gss[:, 1:2], SQRT, bias=epsm, scale=1.0)
                nc.vector.reciprocal(out=g_sb[:, 1:2], in_=stdg)

                # broadcast per-group -> per-channel: [128, 2] = indT8^T @ g_sb
                ppsum = gn_psum.tile([128, 2], F32, tag="ppsum")
                nc.tensor.matmul(ppsum, lhsT=indT8, rhs=g_sb, start=True, stop=True)

                # alpha/beta read ppsum (PSUM): ScalarE IDENT with AP scale
                alpha = small.tile([128, 1], F32, tag="alpha")
                nc.scalar.activation(
                    alpha, ppsum[:, 1:2], IDENT, scale=gnw_sb[:, ct : ct + 1]
                )
                beta = small.tile([128, 1], F32, tag="beta")
                nc.scalar.activation(beta, ppsum[:, 0:1], IDENT, scale=alpha)
                nc.gpsimd.tensor_sub(beta, gnb_sb[:, ct : ct + 1], beta)

                # t8 = alpha*x + beta, straight to fp8
                nc.scalar.activation(t8[:, ct, :], x_t, IDENT, bias=beta, scale=alpha)

        # ---- phase 2: Q / K / V projections (fp8 DoubleRow) ------------
        # A deeper PSUM rotation (5 banks, only possible while the
        # attention pools are closed) keeps matmuls from waiting on
        # eviction drains; evictions alternate DVE / ScalarE
        qkv_ctx = tc.tile_pool(name="qkv_psum", bufs=5, space="PSUM")
        qkv_psum = qkv_ctx.__enter__()
        evict_ctr = [0]

        def evict(dst, src, bias_ap):
            if evict_ctr[0] % 2 == 0:
                if bias_ap is None:
                    nc.vector.tensor_scalar_mul(dst, src, 1.0 / WS)
                else:
                    nc.vector.tensor_scalar(dst, src, 1.0 / WS, bias_ap, op0=MULT, op1=ADD)
            else:
                nc.scalar.activation(
                    dst, src, IDENT,
                    bias=(0.0 if bias_ap is None else bias_ap), scale=1.0 / WS,
                )
            evict_ctr[0] += 1

        for dt in range(CT):
            for nb in range(QB):
                qp = qkv_psum.tile([128, FB], F32, tag="mm")
                for i in range(CT // 2):
                    nc.tensor.matmul(
                        qp,
                        lhsT=w8["wq8"][:, 2 * i : 2 * i + 2, dt * 128 : (dt + 1) * 128],
                        rhs=t8[:, 2 * i : 2 * i + 2, nb * FB : (nb + 1) * FB],
                        start=(i == 0),
                        stop=(i == CT // 2 - 1),
                        perf_mode=DR,
                    )
                evict(q8[:, dt, nb * FB : (nb + 1) * FB], qp, bq_sb[:, dt : dt + 1])
        for dt in range(CT):
            for nb in range(NB):
                kp = qkv_psum.tile([128, FB], F32, tag="mm")
                for i in range(CT // 2):
                    nc.tensor.matmul(
                        kp,
                        lhsT=w8["wk8"][:, 2 * i : 2 * i + 2, dt * 128 : (dt + 1) * 128],
                        rhs=t8[:, 2 * i : 2 * i + 2, nb * FB : (nb + 1) * FB],
                        start=(i == 0),
                        stop=(i == CT // 2 - 1),
                        perf_mode=DR,
                    )
                evict(k8[:, dt, nb * FB : (nb + 1) * FB], kp, bk_sb[:, dt : dt + 1])
        for mt in range(MT):
            vp = qkv_psum.tile([128, C], F32, tag="mm")
            for i in range(CT // 2):
                nc.tensor.matmul(
                    vp,
                    lhsT=t8[:, 2 * i : 2 * i + 2, mt * 128 : (mt + 1) * 128],
                    rhs=w8["wv8"][:, 2 * i : 2 * i + 2, :],
                    start=(i == 0),
                    stop=(i == CT // 2 - 1),
                    perf_mode=DR,
                )
            evict(v8[:, mt, :], vp, None)  # bv folded into bp_eff on host

        qkv_ctx.__exit__(None, None, None)

        # ---- phase 3: attention + proj + residual ----------------------
        with (
            tc.tile_pool(name="o_psum", bufs=1, space="PSUM") as o_psum,
            tc.tile_pool(name="lrb_psum", bufs=1, space="PSUM") as lrb_psum,
            tc.tile_pool(name="e_pool", bufs=6) as e_pool,
            tc.tile_pool(name="att_sb", bufs=3) as att_sb,
            tc.tile_pool(name="y_pool", bufs=6) as y_pool,
        ):
            prev = {}  # qb-1 state: o_sb, rb_sb, qs — projected during qb's S loop

            def deferred_proj(tail=False):
                if not prev:
                    return
                o_prev, rb_prev, qs_prev = prev["o"], prev["rb"], prev["qs"]
                for pt in range(CT):
                    pj = mm_psum.tile([128, FB], F32, tag="mm")
                    for i in range(CT // 2):
                        nc.tensor.matmul(
                            pj,
                            lhsT=w8["wp8"][:, 2 * i : 2 * i + 2, pt * 128 : (pt + 1) * 128],
                            rhs=o_prev[:, 2 * i : 2 * i + 2, :],
                            start=(i == 0),
                            stop=(i == CT // 2 - 1),
                            perf_mode=DR,
                        )
                    y_tile = y_pool.tile([128, FB], F32, tag="y")
                    nc.vector.tensor_mul(y_tile, pj, rb_prev)
                    # Pool's slow tensor_add is fine mid-stream (hidden under
                    # PE) but serializes the kernel tail — use DVE there
                    eng = nc.vector if tail else nc.gpsimd
                    eng.tensor_add(y_tile, y_tile, xb[:, pt, qs_prev])
                    nc.sync.dma_start(
                        out=out_ext[pt * 128 : (pt + 1) * 128, qs_prev], in_=y_tile
                    )
                prev.clear()

            for qb in range(QB):
                qs = slice(qb * FB, (qb + 1) * FB)
                op = [
                    o_psum.tile([128, FB], F32, tag=f"o{dc}", name=f"o_{qb}_{dc}")
                    for dc in range(CT)
                ]
                lp = lrb_psum.tile([128, FB], F32, tag="lrb", name=f"l_{qb}")
                lp1 = lp[0:1, :]

                for pr in range(MT // 2):  # pairs of key tiles
                    etp = e_pool.tile([128, 2, FB], FP8, tag="etp")
                    for half in range(2):
                        mt = 2 * pr + half
                        sp = mm_psum.tile([128, FB], F32, tag="mm")
                        for i in range(CT // 2):
                            nc.tensor.matmul(
                                sp,
                                lhsT=k8[:, 2 * i : 2 * i + 2, mt * 128 : (mt + 1) * 128],
                                rhs=q8[:, 2 * i : 2 * i + 2, qs],
                                start=(i == 0),
                                stop=(i == CT // 2 - 1),
                                perf_mode=DR,
                            )
                        nc.scalar.activation(
                            etp[:, half, :], sp, EXP, bias=expshift, scale=SCALE
                        )
                    nc.tensor.matmul(
                        lp1,
                        lhsT=ones_dr,
                        rhs=etp,
                        start=(pr == 0),
                        stop=(pr == MT // 2 - 1),
                        perf_mode=DR,
                    )
                    for dc in range(CT):
                        nc.tensor.matmul(
                            op[dc],
                            lhsT=v8[:, 2 * pr : 2 * pr + 2, dc * 128 : (dc + 1) * 128],
                            rhs=etp,
                            start=(pr == 0),
                            stop=(pr == MT // 2 - 1),
                            perf_mode=DR,
                        )
                    if pr == 2:
                        deferred_proj()  # project qb-1 while qb's S stream runs

                # rb chain: l_sb = (L/8)/4 in bf16 -> broadcast via ones
                # matmul -> reciprocal => rb = 32/true_L
                l_sb = att_sb.tile([1, FB], BF16, tag="l_sb")
                nc.vector.tensor_scalar_mul(l_sb, lp1, 0.25)
                rbp = lrb_psum.tile([128, FB], F32, tag="lrb", name=f"rb_{qb}")
                nc.tensor.matmul(rbp, lhsT=ones_row, rhs=l_sb, start=True, stop=True)
                rb_sb = att_sb.tile([128, FB], F32, tag="rb_sb")
                nc.vector.reciprocal(out=rb_sb, in_=rbp)

                # evict unnormalized O as fp8 (x 1/64) for the fp8 proj
                o_sb = att_sb.tile([128, CT, FB], FP8, tag="o_sb")
                for dc in range(CT):
                    nc.vector.tensor_scalar_mul(o_sb[:, dc, :], op[dc], 1.0 / 64.0)

                prev.update({"o": o_sb, "rb": rb_sb, "qs": qs})

            deferred_proj(tail=True)  # tail: last query block


_NC_CACHE = None


def _get_nc():
    global _NC_CACHE
    if _NC_CACHE is None:
        _NC_CACHE = _build_nc()
    return _NC_CACHE


def _make_indicators():
    p = np.arange(128)
    ind16 = np.zeros((128, 8), np.float32)
    ind16[p, p // GSIZE] = 1.0 / GSIZE
    indT8 = np.zeros((8, 128), np.float32)
    indT8[p // GSIZE, p] = 1.0
    return ind16, indT8


def _prep_common(inputs):
    """Host-side packing: fp8 weights in DoubleRow pair layout + biases."""
    f8 = ml_dtypes.float8_e4m3fn
    common = {}
    for name, key in (("wq8", "wq"), ("wk8", "wk"), ("wv8", "wv"), ("wp8", "wp")):
        w = np.asarray(inputs[key], np.float32)
        wT16 = np.ascontiguousarray(w.T) * WS                  # [c, d] * 16
        w8 = wT16.astype(f8)                                   # quantize
        # [c, d] -> [128, CT, d] -> [128, CT*d]  (partition = c % 128)
        common[name] = np.ascontiguousarray(
            w8.reshape(CT, 128, C).transpose(1, 0, 2).reshape(128, CT * C)
        )

    wp_q = common["wp8"].reshape(128, CT, C).transpose(1, 0, 2).reshape(C, C)
    wp_deq = wp_q.astype(np.float32) / WS                      # [c, d] = wp.T quantized
    bv = np.asarray(inputs["bv"], np.float64)
    bp_eff = (
        np.asarray(inputs["bp"], np.float64)
        + wp_deq.astype(np.float64).T @ bv                     # wp @ bv
    ).astype(np.float32)
    common["_bp_eff"] = bp_eff  # host-only: folded into the xres staging

    def tile_vec(v):
        return np.ascontiguousarray(np.asarray(v, np.float32).reshape(CT, 128).T)

    biases = np.concatenate(
        [
            tile_vec(inputs["bq"]),
            tile_vec(inputs["bk"]),
            tile_vec(bp_eff),
            tile_vec(inputs["gn_w"]),
            tile_vec(inputs["gn_b"]),
        ],
        axis=1,
    )
    common["biases"] = np.ascontiguousarray(biases)
    common["ind16"], common["indT8"] = _make_indicators()
    return common


def _build_in_maps(inputs):
    x = np.ascontiguousarray(np.asarray(inputs["x"], dtype=np.float32))
    assert x.shape == (B, C, H, W), x.shape
    xf = x.reshape(B, C, N)

    common = _prep_common(inputs)
    bp_eff = common.pop("_bp_eff")

    in_maps = []
    for core in range(8):
        b, h = core // 2, core % 2
        if h == 0:
            xc = xf[b]
        else:
            # rotate so this core's query tokens land in columns 0..NQ-1
            xc = np.concatenate([xf[b][:, NQ:], xf[b][:, :NQ]], axis=1)
        xres = xf[b][:, h * NQ : (h + 1) * NQ] + bp_eff[:, None]
        in_maps.append(
            {
                "x": np.ascontiguousarray(xc),
                "xres": np.ascontiguousarray(xres),
                **common,
            }
        )
    return in_maps


def kernel(**inputs) -> np.ndarray:
    in_maps = _build_in_maps(inputs)
    nc = _get_nc()
    res = run_bass_kernel_spmd(nc, in_maps, core_ids=list(range(8)))

    out = np.empty((B, C, N), np.float32)
    for core in range(8):
        b, h = core // 2, core % 2
        out[b][:, h * NQ : (h + 1) * NQ] = res.results[core]["out"]
    return out.reshape(B, C, H, W)
